# revision 6
# baseline (speedup 1.0000x reference)
"""Cross-modal attention kernel for 8 Trainium2 NeuronCores.

Sharding: pure data parallelism — batch B=8, one batch element per core.
Weights are replicated; no collectives.

Per-core pipeline (every matmul contracts along the SBUF partition dim):
  P1: transpose key_value tiles via PE -> XkvT [d, kv]; project
      Kt[h,kv] = Wk^T XkvT (+bk) and V[kv,h] = XkvT^T Wv (+bv); both are
      spilled to DRAM scratch to bound SBUF residency.
  P2: transpose query tiles -> XqT [d, q]; Qt[h,q] = (Wq^T XqT + bq)/32
      stays resident in SBUF.
  P3: scoresT[kv,q] = Kt^T Qt, evicted from PSUM through a fused ACT op:
      attnT = exp(scoresT + additive_mask).  Scores are O(1) here so exp
      cannot overflow and no row-max subtraction is needed — softmax
      becomes a single fused eviction with no cross-partition reduction.
  P4: row sums via ones-vector matmuls; ctxT[h,q] = V^T attnT directly
      (no context transpose needed); out = ctxT^T Wo with the softmax
      normalization applied as a per-partition PSUM-eviction scale,
      plus bo.
"""

import numpy as np

import concourse.bass as bass
import concourse.mybir as mybir
import concourse.tile as tile
from concourse.bass_utils import run_bass_kernel_spmd
from concourse.tile import ScopedClock

P = 128
LQ, LKV, D, H = 1024, 2048, 1024, 1024
QT, KVT, DT, HT = LQ // P, LKV // P, D // P, H // P  # 8, 16, 8, 8
NCORES = 8
F32 = mybir.dt.float32

# Matmul input dtype: float32r reinterprets fp32 operands in the PE's
# fast path (1 cycle/row at free-dim >= 256 vs 4 cycles/row for fp32).
MM_FAST = True
MMD = mybir.dt.float32r if MM_FAST else mybir.dt.float32

_DRAIN_WAIT_CAP = 1


class _SplitDrainTC(tile.TileContext):
    """Work around this walrus build's 1-wait cap on sync-engine CTRL
    encodings by spreading the final drain's sem waits over nops."""

    def _drain_and_barrier(self, tick_clock, wait_clock):
        drain_inst = self.nc.sync.drain()
        wait_clock.add_sem_waits(
            drain_inst.ins, ScopedClock({None: tick_clock.global_clock})
        )
        si = drain_inst.ins.sync_info
        waits = list(si.on_wait or [])
        if len(waits) > _DRAIN_WAIT_CAP:
            si.on_wait = waits[:_DRAIN_WAIT_CAP]
            for i in range(_DRAIN_WAIT_CAP, len(waits), _DRAIN_WAIT_CAP):
                nop = self.nc.sync.nop(nofuse=True, hint=f"drain_split_{i}")
                nop.ins.sync_info = mybir.SyncInfo(
                    on_wait=waits[i : i + _DRAIN_WAIT_CAP], on_update=[]
                )

        self.nc.all_engine_barrier()
        assert self.sems is not None
        popped = self.nc._tile_sem_poison_stack.pop()
        assert popped is self._sem_poison
        self.nc.clear_and_free_semaphores(list(self.sems.allocated().values()))
        self.nc.all_engine_barrier()


def _split_waits(nc, cap=1):
    """This walrus build rejects instructions carrying more than one sem
    wait ("Too many sync wait commands").  Spread excess waits onto
    same-engine NOPs inserted immediately before the instruction —
    engine queues are FIFO, so the waits still complete first."""
    k = 0
    for f in nc.m.functions:
        for bb in f.blocks:
            insts = bb.instructions
            new = []
            changed = False
            for inst in insts:
                si = inst.sync_info
                waits = list(si.on_wait) if (si and si.on_wait) else []
                if len(waits) > cap:
                    changed = True
                    for i in range(0, len(waits) - cap, cap):
                        nop = mybir.InstNoOp(name=f"waitsplit_{k}", ins=[], outs=[])
                        k += 1
                        nop.engine = inst.engine
                        nop.sync_info = mybir.SyncInfo(
                            on_wait=waits[i : i + cap], on_update=[]
                        )
                        new.append(nop)
                    si.on_wait = waits[len(waits) - cap :]
                new.append(inst)
            if changed:
                bb.instructions = new


def _build_nc():
    nc = bass.Bass("TRN2", debug=False, num_devices=NCORES)

    xq = nc.dram_tensor("xq", [LQ, D], F32, kind="ExternalInput")
    xkv = nc.dram_tensor("xkv", [LKV, D], F32, kind="ExternalInput")
    wq = nc.dram_tensor("wq", [D, H], MMD, kind="ExternalInput")
    wk = nc.dram_tensor("wk", [D, H], MMD, kind="ExternalInput")
    wv = nc.dram_tensor("wv", [D, H], MMD, kind="ExternalInput")
    wo = nc.dram_tensor("wo", [H, D], MMD, kind="ExternalInput")
    # host-prestriped per-partition bias/mask layouts
    bqs = nc.dram_tensor("bqs", [P, HT], F32, kind="ExternalInput")  # bq/32, striped
    bks = nc.dram_tensor("bks", [P, HT], F32, kind="ExternalInput")
    maskb = nc.dram_tensor("maskb", [P, KVT], F32, kind="ExternalInput")
    bvr = nc.dram_tensor("bvr", [P, H], F32, kind="ExternalInput")  # bv replicated
    bor = nc.dram_tensor("bor", [P, D], F32, kind="ExternalInput")  # bo replicated
    ident = nc.dram_tensor("ident", [P, P], F32, kind="ExternalInput")
    ones = nc.dram_tensor("ones", [P, 1], F32, kind="ExternalInput")

    out = nc.dram_tensor("out", [LQ, D], F32, kind="ExternalOutput")

    AF = mybir.ActivationFunctionType

    with _SplitDrainTC(nc) as tc:
        with (
            tc.tile_pool(name="consts", bufs=1) as consts,
            tc.tile_pool(name="psum", bufs=1, space="PSUM") as psum,
            tc.tile_pool(name="dram", bufs=1, space="DRAM") as dram,
        ):
            bqs_t = consts.tile([P, HT], F32)
            nc.sync.dma_start(bqs_t[:], bqs[:, :])
            bks_t = consts.tile([P, HT], F32)
            nc.sync.dma_start(bks_t[:], bks[:, :])
            mask_t = consts.tile([P, KVT], F32)
            nc.sync.dma_start(mask_t[:], maskb[:, :])
            bvr_t = consts.tile([P, H], F32)
            nc.sync.dma_start(bvr_t[:], bvr[:, :])
            bor_t = consts.tile([P, D], F32)
            nc.sync.dma_start(bor_t[:], bor[:, :])
            id_t = consts.tile([P, P], F32)
            nc.sync.dma_start(id_t[:], ident[:, :])
            ones_t = consts.tile([P, 1], F32)
            nc.sync.dma_start(ones_t[:], ones[:, :])

            ktD = dram.tile([HT, P, LKV], MMD)  # Kt[h,kv]: [ht, p, kv]
            vD = dram.tile([KVT, P, H], MMD)  # V[kv,h]: [kvt, p, h]

            wk_view = wk[:, :].rearrange("(t p) h -> p t h", p=P)
            wv_view = wv[:, :].rearrange("(t p) h -> p t h", p=P)
            wq_view = wq[:, :].rearrange("(t p) h -> p t h", p=P)
            wo_view = wo[:, :].rearrange("(t p) h -> p t h", p=P)

            # ---------------- P1: K/V projections ----------------
            with (
                tc.tile_pool(name="wkv", bufs=2) as wkvp,
                tc.tile_pool(name="p1rows", bufs=4) as rowsp,
                tc.tile_pool(name="p1xkvT", bufs=2) as xkvTp,
                tc.tile_pool(name="p1bounce", bufs=4) as bouncep,
            ):
                wk_t = wkvp.tile([P, DT, H], MMD, tag="w")
                nc.sync.dma_start(wk_t[:], wk_view)
                wv_t = wkvp.tile([P, DT, H], MMD, tag="w")
                nc.sync.dma_start(wv_t[:], wv_view)

                NCH = 4  # kv chunks of 512
                CW = LKV // NCH
                CT = CW // P  # 4 row tiles per chunk
                for c in range(NCH):
                    rows = []
                    for s in range(CT):
                        r = rowsp.tile([P, D], F32, tag="rows")
                        nc.sync.dma_start(
                            r[:], xkv[(c * CT + s) * P : (c * CT + s + 1) * P, :]
                        )
                        rows.append(r)
                    xkvT_c = xkvTp.tile([P, DT, CW], MMD, tag="xkvT")
                    for s in range(CT):
                        for dc in range(DT):
                            ps = psum.tile([P, P], F32, tag="tp", bufs=2)
                            nc.tensor.transpose(
                                ps[:], rows[s][:, dc * P : (dc + 1) * P], id_t[:]
                            )
                            nc.vector.tensor_copy(
                                xkvT_c[:, dc, s * P : (s + 1) * P], ps[:]
                            )
                    # Kt chunk
                    for ht in range(HT):
                        pk = psum.tile([P, CW], F32, tag="mm", bufs=6)
                        for dt in range(DT):
                            nc.tensor.matmul(
                                pk[:],
                                wk_t[:, dt, ht * P : (ht + 1) * P],
                                xkvT_c[:, dt, :],
                                start=(dt == 0),
                                stop=(dt == DT - 1),
                            )
                        kb = bouncep.tile([P, CW], MMD, tag="kb")
                        nc.scalar.activation(
                            kb[:], pk[:], AF.Identity, bias=bks_t[:, ht : ht + 1]
                        )
                        nc.sync.dma_start(ktD[ht, :, c * CW : (c + 1) * CW], kb[:])
                    # V chunk
                    for s in range(CT):
                        for hc in range(2):
                            pv = psum.tile([P, 512], F32, tag="mm", bufs=6)
                            for dt in range(DT):
                                nc.tensor.matmul(
                                    pv[:],
                                    xkvT_c[:, dt, s * P : (s + 1) * P],
                                    wv_t[:, dt, hc * 512 : (hc + 1) * 512],
                                    start=(dt == 0),
                                    stop=(dt == DT - 1),
                                )
                            vb = bouncep.tile([P, 512], MMD, tag="vb")
                            nc.vector.tensor_add(
                                vb[:], pv[:], bvr_t[:, hc * 512 : (hc + 1) * 512]
                            )
                            nc.sync.dma_start(
                                vD[c * CT + s, :, hc * 512 : (hc + 1) * 512], vb[:]
                            )

            # attnT spans P3..P4 and must outlive qt (LIFO stack alloc),
            # so its pool opens first.
            with tc.tile_pool(name="attnT", bufs=1) as attnp:
                attnT = attnp.tile([P, KVT, LQ], MMD)

                with tc.tile_pool(name="qt", bufs=1) as qtp:
                    qt_sb = qtp.tile([P, HT, LQ], MMD)

                    # ---------------- P2: Q projection ----------------
                    with (
                        tc.tile_pool(name="wqp", bufs=1) as wqp,
                        tc.tile_pool(name="p2rows", bufs=3) as rowsp2,
                        tc.tile_pool(name="p2xqT", bufs=2) as xqTp,
                    ):
                        wq_t = wqp.tile([P, DT, H], MMD)
                        nc.sync.dma_start(wq_t[:], wq_view)
                        NQC = 2  # q chunks of 512
                        QW = LQ // NQC
                        QCT = QW // P
                        for c in range(NQC):
                            rows = []
                            for s in range(QCT):
                                r = rowsp2.tile([P, D], F32, tag="rows")
                                nc.sync.dma_start(
                                    r[:],
                                    xq[(c * QCT + s) * P : (c * QCT + s + 1) * P, :],
                                )
                                rows.append(r)
                            xqT_c = xqTp.tile([P, DT, QW], MMD, tag="xqT")
                            for s in range(QCT):
                                for dc in range(DT):
                                    ps = psum.tile([P, P], F32, tag="tp", bufs=2)
                                    nc.tensor.transpose(
                                        ps[:],
                                        rows[s][:, dc * P : (dc + 1) * P],
                                        id_t[:],
                                    )
                                    nc.vector.tensor_copy(
                                        xqT_c[:, dc, s * P : (s + 1) * P], ps[:]
                                    )
                            for ht in range(HT):
                                pq = psum.tile([P, QW], F32, tag="mm", bufs=6)
                                for dt in range(DT):
                                    nc.tensor.matmul(
                                        pq[:],
                                        wq_t[:, dt, ht * P : (ht + 1) * P],
                                        xqT_c[:, dt, :],
                                        start=(dt == 0),
                                        stop=(dt == DT - 1),
                                    )
                                nc.scalar.activation(
                                    qt_sb[:, ht, c * QW : (c + 1) * QW],
                                    pq[:],
                                    AF.Identity,
                                    bias=bqs_t[:, ht : ht + 1],
                                    scale=1.0 / 32.0,
                                )

                    # ---------------- P3: scoresT + exp ----------------
                    with tc.tile_pool(name="ksl", bufs=3) as kslp:
                        for kvt in range(KVT):
                            ksl = kslp.tile([P, HT, P], MMD, tag="ksl")
                            nc.sync.dma_start(
                                ksl[:],
                                ktD[:, :, kvt * P : (kvt + 1) * P].rearrange(
                                    "t p k -> p t k"
                                ),
                            )
                            for qc in range(2):
                                ps = psum.tile([P, 512], F32, tag="mm", bufs=6)
                                for ht in range(HT):
                                    nc.tensor.matmul(
                                        ps[:],
                                        ksl[:, ht, :],
                                        qt_sb[:, ht, qc * 512 : (qc + 1) * 512],
                                        start=(ht == 0),
                                        stop=(ht == HT - 1),
                                    )
                                nc.scalar.activation(
                                    attnT[:, kvt, qc * 512 : (qc + 1) * 512],
                                    ps[:],
                                    AF.Exp,
                                    bias=mask_t[:, kvt : kvt + 1],
                                )

                # ---------------- P4: sums, PV (direct ctxT), out ----------------
                with (
                    tc.tile_pool(name="wop", bufs=1) as wop,
                    tc.tile_pool(name="small", bufs=1) as smallp,
                    tc.tile_pool(name="ctxT", bufs=1) as ctxp,
                    tc.tile_pool(name="vts", bufs=2) as vtp,
                    tc.tile_pool(name="ob", bufs=2) as obp,
                ):
                    wo_t = wop.tile([P, HT, D], MMD)
                    nc.sync.dma_start(wo_t[:], wo_view)

                    sums_sb = smallp.tile([P, QT], F32)
                    recip_sb = smallp.tile([P, QT], F32)
                    for qt in range(QT):
                        pss = psum.tile([P, 1], F32, tag="tp", bufs=2)
                        for kvt in range(KVT):
                            nc.tensor.matmul(
                                pss[:],
                                attnT[:, kvt, qt * P : (qt + 1) * P].bitcast(F32),
                                ones_t[:, 0:1],
                                start=(kvt == 0),
                                stop=(kvt == KVT - 1),
                            )
                        nc.vector.tensor_copy(sums_sb[:, qt : qt + 1], pss[:])
                    nc.vector.reciprocal(recip_sb[:], sums_sb[:])

                    ctxT_sb = ctxp.tile([P, HT, LQ], MMD)
                    for ht in range(HT):
                        vts = vtp.tile([P, KVT, P], MMD, tag="vts")
                        nc.sync.dma_start(
                            vts[:],
                            vD[:, :, ht * P : (ht + 1) * P].rearrange("t p h -> p t h"),
                        )
                        for qc in range(2):
                            pc = psum.tile([P, 512], F32, tag="mm", bufs=6)
                            for kvt in range(KVT):
                                nc.tensor.matmul(
                                    pc[:],
                                    vts[:, kvt, :],
                                    attnT[:, kvt, qc * 512 : (qc + 1) * 512],
                                    start=(kvt == 0),
                                    stop=(kvt == KVT - 1),
                                )
                            nc.vector.tensor_copy(
                                ctxT_sb[:, ht, qc * 512 : (qc + 1) * 512], pc[:]
                            )

                    # output projection with fused softmax normalization
                    for qt in range(QT):
                        for dqc in range(2):
                            po = psum.tile([P, 512], F32, tag="mm", bufs=6)
                            for ht in range(HT):
                                nc.tensor.matmul(
                                    po[:],
                                    ctxT_sb[:, ht, qt * P : (qt + 1) * P],
                                    wo_t[:, ht, dqc * 512 : (dqc + 1) * 512],
                                    start=(ht == 0),
                                    stop=(ht == HT - 1),
                                )
                            ob = obp.tile([P, 512], F32, tag="ob")
                            nc.scalar.mul(ob[:], po[:], recip_sb[:, qt : qt + 1])
                            nc.vector.tensor_add(
                                ob[:], ob[:], bor_t[:, dqc * 512 : (dqc + 1) * 512]
                            )
                            nc.sync.dma_start(
                                out[
                                    qt * P : (qt + 1) * P,
                                    dqc * 512 : (dqc + 1) * 512,
                                ],
                                ob[:],
                            )
    _split_waits(nc)
    return nc


_NC_CACHE = {}


def _get_nc():
    if "nc" not in _NC_CACHE:
        _NC_CACHE["nc"] = _build_nc()
    return _NC_CACHE["nc"]


def kernel(query, key_value, key_mask, Wq, bq, Wk, bk, Wv, bv, Wo, bo, **_):
    query = np.asarray(query, dtype=np.float32)
    key_value = np.asarray(key_value, dtype=np.float32)
    key_mask = np.asarray(key_mask)
    Wq = np.asarray(Wq, dtype=np.float32)
    Wk = np.asarray(Wk, dtype=np.float32)
    Wv = np.asarray(Wv, dtype=np.float32)
    Wo = np.asarray(Wo, dtype=np.float32)
    bq = np.asarray(bq, dtype=np.float32)
    bk = np.asarray(bk, dtype=np.float32)
    bv = np.asarray(bv, dtype=np.float32)
    bo = np.asarray(bo, dtype=np.float32)

    B = query.shape[0]
    assert B == NCORES

    # host-side constant prep (negligible cost)
    bqs = (bq / 32.0).reshape(HT, P).T.copy()  # [P, HT]
    bks = bk.reshape(HT, P).T.copy()
    bvr = np.broadcast_to(bv, (P, H)).copy()
    bor = np.broadcast_to(bo, (P, D)).copy()
    ident = np.eye(P, dtype=np.float32)
    ones = np.ones((P, 1), dtype=np.float32)
    # additive mask: 0 where attended, -1e9 where masked
    maskadd = (key_mask.astype(np.float32) - 1.0) * 1e9  # [B, LKV]

    nc = _get_nc()
    in_maps = []
    for b in range(B):
        in_maps.append(
            {
                "xq": np.ascontiguousarray(query[b]),
                "xkv": np.ascontiguousarray(key_value[b]),
                "wq": Wq,
                "wk": Wk,
                "wv": Wv,
                "wo": Wo,
                "bqs": bqs,
                "bks": bks,
                "maskb": np.ascontiguousarray(maskadd[b].reshape(KVT, P).T),
                "bvr": bvr,
                "bor": bor,
                "ident": ident,
                "ones": ones,
            }
        )
    res = run_bass_kernel_spmd(nc, in_maps, core_ids=list(range(NCORES)))
    out_full = np.stack([res.results[b]["out"] for b in range(B)], axis=0)
    return out_full.astype(np.float32)
